# revision 22
# baseline (speedup 1.0000x reference)
"""Trainium2 Bass kernel for nn_DrugRank (GNN message passing), 8 NeuronCores.

Strategy (SPMD, one program on 8 cores):
  - bio graph (50000 nodes, 800000 edges): the branch output is only row
    49999 of layer 2, so the dataflow backward-slices to ~20 layer-1 nodes
    (in-neighbors of 49999) and ~320 layer-0 edges (~320 distinct source
    nodes). Host prep extracts the slice and folds the GCN edge norms into
    two small dense weight matrices:
        A1 = W1mat^T @ (dinv0 * (xV0 @ Wb1))          [128 slots, 200]
        row49999 = w2vec^T @ (dinv1 * relu(...) @ Wb2)
    computed replicated on every core — no gathers, no collectives.
  - cll graph (3451 nodes, padded 4096): dst-node sharding, 512 nodes/core,
    4 blocks of 128 dst slots. Per block: bulk dma_gather of source rows
    from an AllGathered fp16 table, one-hot selection matmul accumulates the
    segment sum in PSUM. Symmetric norm folded as g = dinv*h into the table.
    Layer 4 (200->3) aggregates first, then transforms. The 10353x1000
    dense is row-sharded (aligned with node sharding); partials summed with
    a single AllReduce.
  - mol branch + fusion head: replicated on all cores (tiny).
"""

import numpy as np

import concourse.bacc as bacc
import concourse.bass as bass
import concourse.mybir as mybir
import concourse.tile as tile
from concourse._compat import cdiv
from concourse.bass_utils import run_bass_kernel_spmd

NCORES = 8
F = 200          # GCN feature width
FP = 256         # padded table width (512B fp16 rows for dma_gather)
P = 128

BIO_N = 50000
V0PAD, V0BLK = 640, 5    # padded bio layer-0 node count
CLL_N, CLL_E, CLL_NPAD, CLL_NPC = 3451, 55216, 4096, 512
CLL_NBLK = CLL_NPC // P                 # 4 local dst blocks per core
CLL_TBLK = 27                           # full-table node blocks (27*128=3456)
MOL_N, MOL_E = 64, 128

f32 = mybir.dt.float32
f16 = mybir.dt.float16
f8 = mybir.dt.float8e4
i16 = mybir.dt.int16
RELU = mybir.ActivationFunctionType.Relu
SQRT = mybir.ActivationFunctionType.Sqrt
COPY = mybir.ActivationFunctionType.Copy
EQ = mybir.AluOpType.is_equal
MUL = mybir.AluOpType.mult
ADD = mybir.AluOpType.add


# ---------------------------------------------------------------- host prep

def _pack_idx16(flat):
    """Pack int array (len = multiple of 128) into dma_gather idx layout:
    [128, len/16] int16; idx i at partition i%16, col i//16, tiled x8."""
    n = len(flat)
    a16 = np.asarray(flat, np.int16).reshape(n // 16, 16).T  # [16, n/16]
    return np.ascontiguousarray(np.tile(a16, (8, 1)))


def _pack_slots(flat, dtype=np.float16):
    """[128, len/128]; edge i at partition i%128, col i//128."""
    n = len(flat)
    return np.ascontiguousarray(
        np.asarray(flat, np.float64).astype(dtype).reshape(n // P, P).T)


def _prep_edges(edge, npc, nblk):
    """Bucket edges (with self-loops appended) by dst core, dst-sort,
    block-group.

    Returns per-core packed (idx, slot) arrays, per-block tile counts
    tlo[b] (shared across cores = max, for the SPMD program), and per-core
    degree tiles counted over the REAL edges only."""
    src = edge[0].astype(np.int64)
    dst = edge[1].astype(np.int64)
    nreal = nblk * NCORES * 0 + CLL_N
    aug = np.arange(nreal, dtype=np.int64)
    asrc = np.concatenate([src, aug])
    adst = np.concatenate([dst, aug])
    per_core = []
    degs = []
    for c in range(NCORES):
        sel = (adst >= c * npc) & (adst < (c + 1) * npc)
        s, d = asrc[sel], adst[sel] - c * npc
        order = np.argsort(d, kind="stable")
        s, d = s[order], d[order]
        blocks = []
        for b in range(nblk):
            m = (d >= b * P) & (d < (b + 1) * P)
            blocks.append((s[m], d[m] - b * P))
        per_core.append(blocks)
        rsel = (dst >= c * npc) & (dst < (c + 1) * npc)
        degs.append(np.bincount(dst[rsel] - c * npc,
                                minlength=nblk * P).astype(np.float64))
    tlo = [max(max(cdiv(len(per_core[c][b][0]), P), 1) for c in range(NCORES))
           for b in range(nblk)]
    idxs, slots = [], []
    for c in range(NCORES):
        fi, fs = [], []
        for b in range(nblk):
            seg_s, seg_d = per_core[c][b]
            n = tlo[b] * P
            pi = np.zeros(n, np.int64)
            ps = np.full(n, -1.0, np.float64)
            pi[:len(seg_s)] = seg_s
            ps[:len(seg_d)] = seg_d
            fi.append(pi)
            fs.append(ps)
        fi, fs = np.concatenate(fi), np.concatenate(fs)
        idxs.append(_pack_idx16(fi))
        slots.append(_pack_slots(fs))
    deg_tiles = [np.ascontiguousarray(d.reshape(nblk, P).T.astype(np.float32))
                 for d in degs]
    return idxs, slots, tlo, deg_tiles


def _prep_bio(edge, x_bio):
    """Backward slice of the bio branch from output row 49999."""
    src = edge[0].astype(np.int64)
    dst = edge[1].astype(np.int64)
    deg = np.bincount(dst, minlength=BIO_N).astype(np.float64)
    dinv = 1.0 / np.sqrt(deg + 1.0)

    s2 = src[dst == BIO_N - 1]
    v1 = np.unique(np.concatenate([[BIO_N - 1], s2]))
    assert len(v1) <= P, len(v1)
    m1 = np.isin(dst, v1)
    e1s, e1d = src[m1], dst[m1]
    v0 = np.unique(np.concatenate([v1, e1s]))
    assert len(v0) <= V0PAD, len(v0)
    v0i = {v: i for i, v in enumerate(v0)}
    v1i = {v: i for i, v in enumerate(v1)}

    xv0 = np.zeros((V0PAD, FP), np.float32)
    xv0[:len(v0), :] = x_bio[v0]
    xv0T = np.ascontiguousarray(xv0.T.astype(np.float16))    # [256, 640]

    # src-side dinv is folded into g1 on device, dst-side dinv is applied at
    # evict — so W1mat carries pure edge counts (+1 self), and w2vec carries
    # dinv[49999] * counts (+ dinv[49999] self).
    w1 = np.zeros((V0PAD, P), np.float64)
    np.add.at(w1, (np.array([v0i[s] for s in e1s]),
                   np.array([v1i[d] for d in e1d])), 1.0)
    for v in v1:
        w1[v0i[v], v1i[v]] += 1.0
    w1mat = np.ascontiguousarray(w1.astype(np.float16))      # [640, 128]

    w2 = np.zeros((P, 1), np.float64)
    for s in s2:
        w2[v1i[s], 0] += dinv[BIO_N - 1]
    w2[v1i[BIO_N - 1], 0] += dinv[BIO_N - 1]
    w2vec = np.ascontiguousarray(w2.astype(np.float16))      # [128, 1]

    dv0 = np.zeros((V0PAD,), np.float32)
    dv0[:len(v0)] = dinv[v0]
    dinvV0 = np.ascontiguousarray(dv0.reshape(V0BLK, P).T)   # [128, 5]
    dv1 = np.zeros((P, 1), np.float32)
    dv1[:len(v1), 0] = dinv[v1]
    return xv0T, w1mat, w2vec, dinvV0, dv1


def _col(v):
    return np.ascontiguousarray(np.asarray(v, np.float32).reshape(-1, 1))


def _rep(v, rows=P):
    return np.ascontiguousarray(
        np.tile(np.asarray(v, np.float32).reshape(1, -1), (rows, 1)))


def _btile(v, p, n):
    """bias [p*n] -> [p, n] with column j = v[j*p:(j+1)*p]."""
    return np.ascontiguousarray(
        np.asarray(v, np.float32).reshape(n, p).T)


def prep_inputs(inp):
    """Build per-core in_maps + compile-time meta from the full inputs."""
    meta = {}
    cll_idx, cll_slot, meta["ctlo"], cll_deg = _prep_edges(
        inp["edge_cll"], CLL_NPC, CLL_NBLK)

    xv0T, w1mat, w2vec, dinvV0, dinvV1 = _prep_bio(
        inp["edge_bio"], np.asarray(inp["x_bio"], np.float32))
    bb2 = np.asarray(inp["bb2"], np.float32)
    bb2c = np.zeros((P, 2), np.float32)
    bb2c[:, 0] = bb2[0:128]
    bb2c[0:72, 1] = bb2[128:200]

    mol_s = inp["edge_mol"][0].astype(np.int64)
    mol_d = inp["edge_mol"][1].astype(np.int64)
    order = np.argsort(mol_d, kind="stable")
    mol_idx = _pack_idx16(mol_s[order])
    mol_slot = _pack_slots(mol_d[order].astype(np.float64), np.float32)

    xcT = np.zeros((512, CLL_TBLK * P), np.float32)
    xcT[:, :CLL_N] = inp["x_cll"].T
    deg_full = np.zeros((CLL_TBLK * P,), np.float64)
    deg_full[:CLL_N] = np.bincount(inp["edge_cll"][1].astype(np.int64),
                                   minlength=CLL_N)
    deg_full_t = np.ascontiguousarray(
        deg_full.reshape(CLL_TBLK, P).T.astype(np.float32))   # [128, 27]

    # per-channel row-sliced + zero-padded Wl1c: flat idx = node*3 + ch
    w1c = np.asarray(inp["Wl1c"], np.float32)                      # [3*3451, 1000]
    w1c_ch = np.zeros((3, CLL_NPAD, 1000), np.float16)
    for ch in range(3):
        w1c_ch[ch, :CLL_N] = w1c[ch::3].astype(np.float16)

    iota = np.tile(np.arange(P, dtype=np.float32), (P, 1))
    ident = np.eye(P, dtype=np.float32)
    ones_col = np.ones((P, 1), np.float32)

    shared = {
        "Wb1": np.asarray(inp["Wb1"], np.float16),
        "Wb2": np.asarray(inp["Wb2"], np.float16),
        "bb1_rep": _rep(inp["bb1"]), "bb2c": bb2c,
        "xV0T": xv0T, "W1mat": w1mat, "w2vec": w2vec,
        "dinvV0": dinvV0, "dinvV1": dinvV1,
        "Wc1": np.asarray(inp["Wc1"], np.float16),
        "Wc2": np.asarray(inp["Wc2"], np.float16),
        "Wc3": np.asarray(inp["Wc3"], np.float16),
        "Wc4": np.asarray(inp["Wc4"], np.float16),
        "bc1_rep": _rep(inp["bc1"]), "bc2_rep": _rep(inp["bc2"]),
        "bc3_rep": _rep(inp["bc3"]), "bc4_rep": _rep(inp["bc4"]),
        "x_mol": np.asarray(inp["x_mol"], np.float32),
        "xmolT": np.ascontiguousarray(inp["x_mol"].T.astype(np.float32)),
        "mol_idx": mol_idx, "mol_slot": mol_slot,
        "Wm1r": np.asarray(inp["Wm1r"], np.float32),
        "Wm1s": np.asarray(inp["Wm1s"], np.float32),
        "Wm2r": np.asarray(inp["Wm2r"], np.float32),
        "Wm2s": np.asarray(inp["Wm2s"], np.float32),
        "bm1_rep": _rep(inp["bm1"]), "bm2_rep": _rep(inp["bm2"]),
        "Wlm": np.asarray(inp["Wlm"], np.float32), "blm_col": _col(inp["blm"]),
        "Wlb": np.asarray(inp["Wlb"], np.float32), "blb_col": _col(inp["blb"]),
        "Wd1": np.asarray(inp["Wd1"], np.float32), "bd1_t": _btile(inp["bd1"], 125, 4),
        "Wd2": np.asarray(inp["Wd2"], np.float32), "bd2_t": _btile(inp["bd2"], 128, 2),
        "Wcat1": np.asarray(inp["Wcat1"], np.float32),
        "bcat1_t": _btile(inp["bcat1"], 125, 8),
        "Wcat2": np.asarray(inp["Wcat2"], np.float32),
        "bcat2_t": np.asarray(inp["bcat2"], np.float32).reshape(1, 1),
        "bl1c_t": _btile(inp["bl1c"], 125, 8),
        "Wl2c": np.asarray(inp["Wl2c"], np.float32),
        "bl2c_t": _btile(inp["bl2c"], 125, 8),
        "Wl3c": np.asarray(inp["Wl3c"], np.float32),
        "bl3c_t": _btile(inp["bl3c"], 128, 2),
        "iota32": iota, "iota16": iota.astype(np.float16),
        "ident32": ident, "ident16": ident.astype(np.float16),
        "ones16": ones_col.astype(np.float16), "ones32": ones_col,
        "xcllT": np.ascontiguousarray(xcT.astype(np.float16)),
        "cll_degf": deg_full_t,
    }
    in_maps = []
    for c in range(NCORES):
        m = dict(shared)
        m["cll_idx"] = cll_idx[c]
        m["cll_slot"] = cll_slot[c]
        m["cll_deg"] = cll_deg[c]
        m["W1c_ch"] = np.ascontiguousarray(
            w1c_ch[:, c * CLL_NPC:(c + 1) * CLL_NPC, :])
        in_maps.append(m)
    return in_maps, meta


# ------------------------------------------------------------ device program

RG = [list(range(NCORES))]


def _declare_inputs(nc, meta):
    tc_ = sum(meta["ctlo"])
    spec = {
        "Wb1": ([256, F], f16), "Wb2": ([F, F], f16),
        "bb1_rep": ([P, F], f32), "bb2c": ([P, 2], f32),
        "xV0T": ([FP, V0PAD], f16), "W1mat": ([V0PAD, P], f16),
        "w2vec": ([P, 1], f16),
        "dinvV0": ([P, V0BLK], f32), "dinvV1": ([P, 1], f32),
        "xcllT": ([512, CLL_TBLK * P], f16),
        "cll_idx": ([P, 8 * tc_], i16), "cll_slot": ([P, tc_], f16),
        "cll_deg": ([P, CLL_NBLK], f32),
        "cll_degf": ([P, CLL_TBLK], f32),
        "Wc1": ([512, F], f16), "Wc2": ([F, F], f16), "Wc3": ([F, F], f16),
        "Wc4": ([F, 3], f16),
        "bc1_rep": ([P, F], f32), "bc2_rep": ([P, F], f32),
        "bc3_rep": ([P, F], f32), "bc4_rep": ([P, 3], f32),
        "W1c_ch": ([3, CLL_NPC, 1000], f16),
        "x_mol": ([MOL_N, 64], f32), "xmolT": ([64, MOL_N], f32),
        "mol_idx": ([P, 8], i16), "mol_slot": ([P, 1], f32),
        "Wm1r": ([64, F], f32), "Wm1s": ([64, F], f32),
        "Wm2r": ([F, F], f32), "Wm2s": ([F, F], f32),
        "bm1_rep": ([P, F], f32), "bm2_rep": ([P, F], f32),
        "Wlm": ([F, 128], f32), "blm_col": ([128, 1], f32),
        "Wlb": ([F, 128], f32), "blb_col": ([128, 1], f32),
        "Wd1": ([256, 500], f32), "bd1_t": ([125, 4], f32),
        "Wd2": ([500, 256], f32), "bd2_t": ([128, 2], f32),
        "Wcat1": ([512, 1000], f32), "bcat1_t": ([125, 8], f32),
        "Wcat2": ([1000, 1], f32), "bcat2_t": ([1, 1], f32),
        "bl1c_t": ([125, 8], f32),
        "Wl2c": ([1000, 1000], f32), "bl2c_t": ([125, 8], f32),
        "Wl3c": ([1000, 256], f32), "bl3c_t": ([128, 2], f32),
        "iota32": ([P, P], f32), "iota16": ([P, P], f16),
        "ident32": ([P, P], f32), "ident16": ([P, P], f16),
        "ones16": ([P, 1], f16), "ones32": ([P, 1], f32),
    }
    return {k: nc.dram_tensor(k, s, d, kind="ExternalInput")
            for k, (s, d) in spec.items()}


def build_program(meta, repeat=1):
    nc = bacc.Bacc("TRN2", target_bir_lowering=False, debug=False,
                   enable_asserts=False, num_devices=NCORES,
                   num_swdge_queues=4)
    io = _declare_inputs(nc, meta)
    out = nc.dram_tensor("out", [1, 1], f32, kind="ExternalOutput")

    gc_slice = [nc.dram_tensor(f"gc{l}_slice", [CLL_NPC, FP], f8,
                               kind="Internal") for l in range(1, 4)]
    gc_slice = [None] + gc_slice
    gc_full = [nc.dram_tensor("gc0_full", [CLL_NPAD, FP], f8,
                              kind="Internal")]
    gc_full += [nc.dram_tensor(f"gc{l}_full", [CLL_NPAD, FP], f8,
                               kind="Internal", addr_space="Shared")
                for l in range(1, 4)]
    m1_dram = nc.dram_tensor("m1_dram", [MOL_N, FP], f32, kind="Internal")
    ar_in = nc.dram_tensor("ar_in", [1000], f32, kind="Internal")
    ar_out = nc.dram_tensor("ar_out", [1000], f32, kind="Internal",
                            addr_space="Shared")

    with tile.TileContext(nc) as tc:
        for _ in range(repeat):
            _build(nc, tc, meta, io, out, gc_slice, gc_full, m1_dram,
                   ar_in, ar_out)
    nc.compile()
    return nc


def _build(nc, tc, meta, io, out, gc_slice, gc_full, m1_dram, ar_in, ar_out):
    ctlo = meta["ctlo"]

    with (
        tc.tile_pool(name="const", bufs=1) as cp,
        tc.tile_pool(name="wp", bufs=1) as wp,
        tc.tile_pool(name="big", bufs=1) as bigp,
        tc.tile_pool(name="gs", bufs=2) as gsp,
        tc.tile_pool(name="ct", bufs=2) as ctp,
        tc.tile_pool(name="sb", bufs=3) as sb,
        tc.tile_pool(name="vp", bufs=3) as vp,
        tc.tile_pool(name="mp", bufs=2) as mp,
        tc.tile_pool(name="psE", bufs=3, space="PSUM") as psE,
        tc.tile_pool(name="psT", bufs=2, space="PSUM") as psT,
        tc.tile_pool(name="psM", bufs=2, space="PSUM") as psM,
    ):
        def load(pool, name, rows=None, cols=None, tag=None):
            src = io[name]
            r = rows if rows is not None else src.shape[0]
            c = cols if cols is not None else src.shape[1]
            t = pool.tile([r, c], src.dtype, tag=tag or name)
            nc.sync.dma_start(t[:], src[0:r, 0:c])
            return t

        iota16 = load(cp, "iota16")
        iota32 = load(cp, "iota32")
        ident16 = load(cp, "ident16")
        ident32 = load(cp, "ident32")

        qrr = [0]

        def next_q():
            qrr[0] = (qrr[0] + 1) % 4
            return qrr[0]

        def onehot_block(sl, tb):
            """Whole block one-hot [128 edges, tb, 128 slots] in one DVE op."""
            mblk = mp.tile([P, tb, P], f8, tag="M", name="mblk")
            nc.vector.tensor_tensor(
                mblk[:], iota16[:, None, 0:P].broadcast_to([P, tb, P]),
                sl[:, :, None].broadcast_to([P, tb, P]), op=EQ)
            return mblk

        # local dinv = 1/sqrt(deg+1) from host-counted real-edge degrees
        def mk_dinv(key, nblk, tag):
            degt = load(cp, key)
            d = cp.tile([P, nblk], f32, tag=tag, name=tag)
            nc.vector.tensor_scalar(d[:], degt[:], 1.0, None, op0=ADD)
            nc.vector.reciprocal(d[:], d[:])
            nc.scalar.activation(d[:], d[:], SQRT)
            return d

        dinv_c = mk_dinv("cll_deg", CLL_NBLK, "dinvc")
        dinv_cf = mk_dinv("cll_degf", CLL_TBLK, "dinvcf")

        # ---------------- shared phase-A / edge-pass helpers ----------------
        def phase_scale_store(ps, j, dinv, dram, row0):
            st = sb.tile([P, FP], f8, tag="fp8st")
            nc.vector.tensor_scalar(st[:, 0:F], ps[:],
                                    dinv[:, j:j + 1], None, op0=MUL)
            nc.sync.dma_start(dram[row0:row0 + P, 0:FP], st[:])

        def allgather(src, dst):
            nc.gpsimd.collective_compute(
                "AllGather", mybir.AluOpType.bypass, replica_groups=RG,
                ins=[src.ap()], outs=[dst.ap()])

        def edge_pass(nblk, tlo, idx_dram, slot_dram, lo_ap, evict):
            tbase = 0
            for b in range(nblk):
                tb = tlo[b]
                sl = sb.tile([P, tb], f16, tag="slotb", name="sl")
                nc.sync.dma_start(sl[:], slot_dram[:, tbase:tbase + tb])
                ix = sb.tile([P, tb * 8], i16, tag="idxb", name="ix")
                nc.sync.dma_start(ix[:], idx_dram[:, tbase * 8:(tbase + tb) * 8])
                ps = psE.tile([P, F], f32, tag="eacc", space="PSUM")
                v = vp.tile([P, max(tb, 1), FP], f8, tag="v")
                for off in range(0, tb, 8):      # <=1024 idxs per gather
                    n = min(8, tb - off)
                    nc.gpsimd.dma_gather(
                        v[:, off:off + n, :], lo_ap,
                        ix[:, off * 8:(off + n) * 8], n * P, n * P, FP,
                        queue_num=next_q())
                mblk = onehot_block(sl, tb)
                for t in range(tb):
                    nc.tensor.matmul(ps[:], mblk[:, t, :], v[:, t, 0:F],
                                     start=(t == 0), stop=(t == tb - 1))
                evict(b, ps)
                tbase += tb

        def evict_common(b, ps, w, dinv, brep, out_sb):
            t2 = sb.tile([P, F], f32, tag="ev1")
            nc.vector.tensor_scalar(t2[:w], ps[:w], dinv[:w, b:b + 1],
                                    None, op0=MUL)
            t3 = sb.tile([P, F], f32, tag="ev2")
            nc.vector.tensor_tensor(t3[:w], t2[:w], brep[:w], op=ADD)
            nc.scalar.activation(out_sb[:w], t3[:w], RELU)

        def transpose_to(src_sb, w, dst0, dst1, b):
            pt = psT.tile([P, P], f32, tag="tp", space="PSUM")
            nc.tensor.transpose(pt[0:P, 0:w], src_sb[:w, 0:P],
                                ident32[:w, :w])
            nc.vector.tensor_copy(dst0[:, b * P:b * P + w], pt[0:P, 0:w])
            pt2 = psT.tile([P, P], f32, tag="tp", space="PSUM")
            nc.tensor.transpose(pt2[0:F - P, 0:w], src_sb[:w, P:F],
                                ident32[:w, :w])
            nc.vector.tensor_copy(dst1[0:F - P, b * P:b * P + w],
                                  pt2[0:F - P, 0:w])

        # -------- cll phase A layer 1: full table, replicated (no AG) --------
        wc1 = [wp.tile([P, F], f16, tag=f"wc1_{k}", name=f"wc1_{k}")
               for k in range(4)]
        for k in range(4):
            nc.sync.dma_start(wc1[k][:], io["Wc1"][k * P:(k + 1) * P, :])
        xc = [bigp.tile([P, CLL_TBLK * P], f16, tag=f"xc{k}", name=f"xc{k}")
              for k in range(4)]
        for k in range(4):
            nc.sync.dma_start(xc[k][:], io["xcllT"][k * P:(k + 1) * P, :])
        for j in range(CLL_TBLK):
            ps = psM.tile([P, F], f32, tag="misc", space="PSUM")
            for k in range(4):
                nc.tensor.matmul(ps[:], xc[k][:, j * P:(j + 1) * P], wc1[k][:],
                                 start=(k == 0), stop=(k == 3))
            phase_scale_store(ps, j, dinv_cf, gc_full[0], j * P)

        # ---------------- bio micro-branch (replicated, tiny) ----------------
        wb1a = load(wp, "Wb1", rows=128, tag="wb1a")
        wb1b = wp.tile([P, F], f16, tag="wb1b")
        nc.sync.dma_start(wb1b[:], io["Wb1"][128:256, :])
        wb2a = load(wp, "Wb2", rows=128, tag="wb2a")
        wb2b = wp.tile([P, F], f16, tag="wb2b")
        nc.sync.dma_start(wb2b[0:72, :], io["Wb2"][128:200, :])
        bb1r = load(wp, "bb1_rep")
        bb2c = load(wp, "bb2c")
        dinvV0 = load(cp, "dinvV0")
        dinvV1 = load(cp, "dinvV1")
        xv0 = load(wp, "xV0T", rows=128, tag="xv0a")
        xv1 = wp.tile([P, V0PAD], f16, tag="xv0b")
        nc.sync.dma_start(xv1[:], io["xV0T"][128:256, :])
        w1t = [wp.tile([P, P], f16, tag=f"w1m_{b}", name=f"w1m_{b}")
               for b in range(V0BLK)]
        for b in range(V0BLK):
            nc.sync.dma_start(w1t[b][:], io["W1mat"][b * P:(b + 1) * P, :])
        w2v = load(wp, "w2vec")

        g1b = sb.tile([P, V0BLK * F], f16, tag="g1bio")
        for b in range(V0BLK):
            psb = psM.tile([P, F], f32, tag="misc", space="PSUM")
            nc.tensor.matmul(psb[:], xv0[:, b * P:(b + 1) * P], wb1a[:],
                             start=True, stop=False)
            nc.tensor.matmul(psb[:], xv1[:, b * P:(b + 1) * P], wb1b[:],
                             start=False, stop=True)
            nc.vector.tensor_scalar(g1b[:, b * F:(b + 1) * F], psb[:],
                                    dinvV0[:, b:b + 1], None, op0=MUL)
        psA1 = psM.tile([P, F], f32, tag="misc", space="PSUM")
        for b in range(V0BLK):
            nc.tensor.matmul(psA1[:], w1t[b][:], g1b[:, b * F:(b + 1) * F],
                             start=(b == 0), stop=(b == V0BLK - 1))
        bgr1 = sb.tile([P, F], f32, tag="bgr1")
        evict_common(0, psA1, P, dinvV1, bb1r, bgr1)
        g2in = sb.tile([P, F], f16, tag="g2in")
        nc.vector.tensor_scalar(g2in[:], bgr1[:], dinvV1[:, 0:1],
                                None, op0=MUL)
        # wvec_gT [200,1] = g2in^T @ w2vec
        wg = sb.tile([P, 2], f16, tag="wgT")
        psw0 = psM.tile([P, 1], f32, tag="misc", space="PSUM")
        nc.tensor.matmul(psw0[:], g2in[:, 0:P], w2v[:], start=True, stop=True)
        nc.vector.tensor_copy(wg[:, 0:1], psw0[:])
        psw1 = psM.tile([F - P, 1], f32, tag="misc", space="PSUM")
        nc.tensor.matmul(psw1[:], g2in[:, P:F], w2v[:], start=True, stop=True)
        nc.vector.tensor_copy(wg[0:F - P, 1:2], psw1[:])
        # A2T = Wb2^T @ wvec_g, as two column blocks; relu(+bb2) -> bgc0/bgc1
        bgc0 = sb.tile([P, 1], f32, tag="bgc0")
        bgc1 = sb.tile([P, 1], f32, tag="bgc1")
        psa = psM.tile([P, 1], f32, tag="misc", space="PSUM")
        nc.tensor.matmul(psa[:], wb2a[:, 0:P], wg[:, 0:1],
                         start=True, stop=False)
        nc.tensor.matmul(psa[:], wb2b[0:F - P, 0:P], wg[0:F - P, 1:2],
                         start=False, stop=True)
        nc.scalar.activation(bgc0[:], psa[:], RELU, bias=bb2c[:, 0:1])
        psb2 = psM.tile([F - P, 1], f32, tag="misc", space="PSUM")
        nc.tensor.matmul(psb2[:], wb2a[:, P:F], wg[:, 0:1],
                         start=True, stop=False)
        nc.tensor.matmul(psb2[:], wb2b[0:F - P, P:F], wg[0:F - P, 1:2],
                         start=False, stop=True)
        nc.scalar.activation(bgc1[0:F - P, :], psb2[:], RELU,
                             bias=bb2c[0:F - P, 1:2])

        # prefetch dense-1 weights (f16) — stream during cll AGs
        w1cpre = []
        for i, (ch, kt) in enumerate([(c_, k_) for c_ in range(3)
                                      for k_ in range(4)]):
            wt = bigp.tile([P, 1000], f16, tag=f"w1c_{i}", name=f"w1c_{i}")
            nc.sync.dma_start(wt[:], io["W1c_ch"][ch, kt * P:(kt + 1) * P, :])
            w1cpre.append(wt)

        # ---------------- cll layers 1-4 ----------------
        wc2a = load(wp, "Wc2", rows=128, tag="wc2a")
        wc2b = wp.tile([P, F], f16, tag="wc2b")
        nc.sync.dma_start(wc2b[0:72, :], io["Wc2"][128:200, :])
        wc3a = load(wp, "Wc3", rows=128, tag="wc3a")
        wc3b = wp.tile([P, F], f16, tag="wc3b")
        nc.sync.dma_start(wc3b[0:72, :], io["Wc3"][128:200, :])
        bc_rep = [load(wp, f"bc{l}_rep") for l in (1, 2, 3)]

        cT0 = cT1 = None
        for layer in range(3):          # cll GCN layers 1..3 edge+evict
            nT0 = nT1 = None
            if layer < 2:
                nT0 = ctp.tile([P, CLL_NPC], f16, tag="ccT0")
                nT1 = ctp.tile([P, CLL_NPC], f16, tag="ccT1")

            if layer > 0:
                wa, wb = (wc2a, wc2b) if layer == 1 else (wc3a, wc3b)
                for j in range(CLL_NBLK):
                    ps = psM.tile([P, F], f32, tag="misc", space="PSUM")
                    nc.tensor.matmul(ps[:], cT0[:, j * P:(j + 1) * P], wa[:],
                                     start=True, stop=False)
                    nc.tensor.matmul(ps[:], cT1[0:72, j * P:(j + 1) * P],
                                     wb[0:72, :], start=False, stop=True)
                    phase_scale_store(ps, j, dinv_c, gc_slice[layer], j * P)
                allgather(gc_slice[layer], gc_full[layer])

            def evict_cll(b, ps, layer=layer, nT0=nT0, nT1=nT1):
                bg = sb.tile([P, F], f32, tag="ev3")
                evict_common(b, ps, P, dinv_c, bc_rep[layer], bg)
                if layer < 2:
                    transpose_to(bg, P, nT0, nT1, b)
                else:
                    st = sb.tile([P, FP], f8, tag="fp8st")
                    nc.vector.tensor_scalar(st[:, 0:F], bg[:],
                                            dinv_c[:, b:b + 1], None, op0=MUL)
                    nc.sync.dma_start(gc_slice[3][b * P:(b + 1) * P, 0:FP],
                                      st[:])

            edge_pass(CLL_NBLK, ctlo, io["cll_idx"], io["cll_slot"],
                      gc_full[layer].ap(), evict_cll)
            cT0, cT1 = nT0, nT1
        allgather(gc_slice[3], gc_full[3])

        # cll layer 4: aggregate gc4, then transform by Wc4
        agT0 = ctp.tile([P, CLL_NPC], f16, tag="ccT0")
        agT1 = ctp.tile([P, CLL_NPC], f16, tag="ccT1")

        def evict_cll4(b, ps):
            ag = sb.tile([P, F], f32, tag="ev3")
            nc.vector.tensor_scalar(ag[:], ps[:], dinv_c[:, b:b + 1],
                                    None, op0=MUL)
            transpose_to(ag, P, agT0, agT1, b)

        edge_pass(CLL_NBLK, ctlo, io["cll_idx"], io["cll_slot"],
                  gc_full[3].ap(), evict_cll4)

        # ---------------- cll layer 4 transform: h4 = relu(agg4@Wc4+bc4) -----
        wc4a = load(wp, "Wc4", rows=128, tag="wc4a")
        wc4b = wp.tile([P, 3], f16, tag="wc4b")
        nc.sync.dma_start(wc4b[0:72, :], io["Wc4"][128:200, :])
        bc4r = load(wp, "bc4_rep")
        h4_all = sb.tile([P, 12], f16, tag="h4")
        for j in range(CLL_NBLK):
            psh = psM.tile([P, 3], f32, tag="misc", space="PSUM")
            nc.tensor.matmul(psh[:], agT0[:, j * P:(j + 1) * P], wc4a[:],
                             start=True, stop=False)
            nc.tensor.matmul(psh[:], agT1[0:72, j * P:(j + 1) * P],
                             wc4b[0:72, :], start=False, stop=True)
            th = sb.tile([P, 3], f32, tag="th4")
            nc.vector.tensor_tensor(th[:], psh[:], bc4r[:, 0:3], op=ADD)
            nc.scalar.activation(h4_all[:, j * 3:(j + 1) * 3], th[:], RELU)

        # ---------------- cll dense-1 row-sharded partials ----------------
        # Each (ch, ktile) step runs 8 independent single matmuls into one
        # PSUM tile's columns, then DVE-accumulates into SBUF (avoids
        # overlapping PSUM accumulation groups in one bank).
        acc_d = sb.tile([125, 8], f32, tag="accd")
        steps = [(c, k) for c in range(3) for k in range(4)]
        for i, (ch, kt) in enumerate(steps):
            wt = w1cpre[i]
            pst = psM.tile([125, 8], f32, tag="misc", space="PSUM")
            for och in range(8):
                nc.tensor.matmul(pst[:, och:och + 1],
                                 wt[:, och * 125:(och + 1) * 125],
                                 h4_all[:, kt * 3 + ch:kt * 3 + ch + 1],
                                 start=True, stop=True)
            if i == 0:
                nc.vector.tensor_copy(acc_d[:], pst[:])
            else:
                nc.vector.tensor_tensor(acc_d[:], acc_d[:], pst[:], op=ADD)
        for j in range(8):
            nc.sync.dma_start(ar_in.ap()[j * 125:(j + 1) * 125, None],
                              acc_d[:, j:j + 1])

        # ---------------- mol branch (replicated, tiny) ----------------
        mol_idx_sb = load(cp, "mol_idx")
        mol_slot_sb = load(cp, "mol_slot")
        xmolT_sb = load(wp, "xmolT")
        wm1r = load(wp, "Wm1r")
        wm1s = load(wp, "Wm1s")
        bm1r = load(wp, "bm1_rep")
        bm2r = load(wp, "bm2_rep")
        v1 = sb.tile([P, 1, 64], f32, tag="vm")
        nc.gpsimd.dma_gather(v1[:], io["x_mol"].ap(), mol_idx_sb[:],
                             MOL_E, MOL_E, 64)
        mM = mp.tile([P, 64], f32, tag="Mmol")
        nc.vector.tensor_scalar(mM[:], iota32[:, 0:64], mol_slot_sb[:, 0:1],
                                None, op0=EQ)
        agg_ps = psM.tile([64, 64], f32, tag="misc", space="PSUM")
        nc.tensor.matmul(agg_ps[:], mM[:], v1[:, 0, :], start=True, stop=True)
        agg_sb = sb.tile([64, 64], f32, tag="mol1")
        nc.vector.tensor_copy(agg_sb[:], agg_ps[:])
        pt = psT.tile([P, P], f32, tag="tp", space="PSUM")
        nc.tensor.transpose(pt[0:64, 0:64], agg_sb[0:64, 0:64],
                            ident32[0:64, 0:64])
        aggT = sb.tile([64, 64], f32, tag="mol2")
        nc.vector.tensor_copy(aggT[:], pt[0:64, 0:64])
        h1_ps = psM.tile([64, F], f32, tag="misc", space="PSUM")
        nc.tensor.matmul(h1_ps[:], aggT[:], wm1r[:], start=True, stop=False)
        nc.tensor.matmul(h1_ps[:], xmolT_sb[:], wm1s[:], start=False, stop=True)
        t_m1 = sb.tile([64, F], f32, tag="mol3")
        nc.vector.tensor_tensor(t_m1[:], h1_ps[:], bm1r[0:64, :], op=ADD)
        m1_sb = sb.tile([64, F], f32, tag="mol4")
        nc.scalar.activation(m1_sb[:], t_m1[:], RELU)
        nc.sync.dma_start(m1_dram[0:64, 0:F], m1_sb[:])

        wm2r0 = load(wp, "Wm2r", rows=128, tag="wm2r0")
        wm2r1 = wp.tile([P, F], f32, tag="wm2r1")
        nc.sync.dma_start(wm2r1[0:72, :], io["Wm2r"][128:200, :])
        wm2s0 = load(wp, "Wm2s", rows=128, tag="wm2s0")
        wm2s1 = wp.tile([P, F], f32, tag="wm2s1")
        nc.sync.dma_start(wm2s1[0:72, :], io["Wm2s"][128:200, :])
        v2 = sb.tile([P, 1, FP], f32, tag="vm2")
        nc.gpsimd.dma_gather(v2[:], m1_dram.ap(), mol_idx_sb[:],
                             MOL_E, MOL_E, FP)
        agg2_ps = psM.tile([64, F], f32, tag="misc", space="PSUM")
        nc.tensor.matmul(agg2_ps[:], mM[:], v2[:, 0, 0:F], start=True, stop=True)
        agg2_sb = sb.tile([64, F], f32, tag="mol1")
        nc.vector.tensor_copy(agg2_sb[:], agg2_ps[:])
        a2T0 = sb.tile([P, 64], f32, tag="mol5")
        a2T1 = sb.tile([P, 64], f32, tag="mol6")
        m1T0 = sb.tile([P, 64], f32, tag="mol7")
        m1T1 = sb.tile([P, 64], f32, tag="mol8")
        for srcT, d0, d1 in ((agg2_sb, a2T0, a2T1), (m1_sb, m1T0, m1T1)):
            pt1 = psT.tile([P, P], f32, tag="tp", space="PSUM")
            nc.tensor.transpose(pt1[0:P, 0:64], srcT[0:64, 0:P],
                                ident32[0:64, 0:64])
            nc.vector.tensor_copy(d0[:, 0:64], pt1[0:P, 0:64])
            pt2 = psT.tile([P, P], f32, tag="tp", space="PSUM")
            nc.tensor.transpose(pt2[0:72, 0:64], srcT[0:64, P:F],
                                ident32[0:64, 0:64])
            nc.vector.tensor_copy(d1[0:72, 0:64], pt2[0:72, 0:64])
        h2_ps = psM.tile([64, F], f32, tag="misc", space="PSUM")
        nc.tensor.matmul(h2_ps[:], a2T0[:, 0:64], wm2r0[:],
                         start=True, stop=False)
        nc.tensor.matmul(h2_ps[:], a2T1[0:72, 0:64], wm2r1[0:72, :],
                         start=False, stop=False)
        nc.tensor.matmul(h2_ps[:], m1T0[:, 0:64], wm2s0[:],
                         start=False, stop=False)
        nc.tensor.matmul(h2_ps[:], m1T1[0:72, 0:64], wm2s1[0:72, :],
                         start=False, stop=True)
        t_m2 = sb.tile([64, F], f32, tag="mol3")
        nc.vector.tensor_tensor(t_m2[:], h2_ps[:], bm2r[0:64, :], op=ADD)
        m2_sb = sb.tile([64, F], f32, tag="mol4")
        nc.scalar.activation(m2_sb[:], t_m2[:], RELU)

        ones32_sb = load(cp, "ones32")
        wlm0 = load(wp, "Wlm", rows=128, tag="wlm0")
        wlm1 = wp.tile([P, 128], f32, tag="wlm1")
        nc.sync.dma_start(wlm1[0:72, :], io["Wlm"][128:200, :])
        blm = load(wp, "blm_col")
        mcol0 = sb.tile([P, 1], f32, tag="mc0")
        mcol1 = sb.tile([P, 1], f32, tag="mc1")
        pool_ps = psM.tile([P, 1], f32, tag="misc", space="PSUM")
        nc.tensor.matmul(pool_ps[0:P, :], m2_sb[0:64, 0:P], ones32_sb[0:64, :],
                         start=True, stop=True)
        nc.scalar.activation(mcol0[:], pool_ps[0:P, :], COPY, scale=1.0 / 64.0)
        pool_ps2 = psM.tile([P, 1], f32, tag="misc", space="PSUM")
        nc.tensor.matmul(pool_ps2[0:72, :], m2_sb[0:64, P:F], ones32_sb[0:64, :],
                         start=True, stop=True)
        nc.scalar.activation(mcol1[0:72, :], pool_ps2[0:72, :], COPY,
                             scale=1.0 / 64.0)
        mvec = sb.tile([P, 1], f32, tag="mvec")
        mm_ps = psM.tile([P, 1], f32, tag="misc", space="PSUM")
        nc.tensor.matmul(mm_ps[:], wlm0[:], mcol0[:], start=True, stop=False)
        nc.tensor.matmul(mm_ps[:], wlm1[0:72, :], mcol1[0:72, :],
                         start=False, stop=True)
        nc.scalar.activation(mvec[:], mm_ps[:], RELU, bias=blm[:])

        # ---------------- head weight prefetch (overlaps AR) ----------
        wtc2 = []
        for k in range(8):
            wt = bigp.tile([125, 1000], f32, tag=f"hw2_{k}", name=f"hw2_{k}")
            nc.sync.dma_start(wt[:], io["Wl2c"][k * 125:(k + 1) * 125, :])
            wtc2.append(wt)
        wtc3 = []
        for k in range(8):
            wt = bigp.tile([125, 256], f32, tag=f"hw3_{k}", name=f"hw3_{k}")
            nc.sync.dma_start(wt[:], io["Wl3c"][k * 125:(k + 1) * 125, :])
            wtc3.append(wt)
        wtu = []
        for k in range(4):
            wt = bigp.tile([P, 1000], f32, tag=f"hwu_{k}", name=f"hwu_{k}")
            nc.sync.dma_start(wt[:], io["Wcat1"][k * P:(k + 1) * P, :])
            wtu.append(wt)

        # ---------------- AllReduce (dense partials) ----------
        nc.gpsimd.collective_compute(
            "AllReduce", mybir.AluOpType.add, replica_groups=RG,
            ins=[ar_in.ap()], outs=[ar_out.ap()])

        # ---------------- fusion head (replicated) ----------------
        def mm_chain(p_rows, n_cols, k_steps, act_bias, out_tag):
            """acc = sum_k (lhsT_k.T @ rhs_k) per column; relu(acc+bias)."""
            acc = sb.tile([p_rows, n_cols], f32, tag=out_tag + "a")
            for k in range(k_steps):
                lhsT, rhs = yield k
                pst = psM.tile([p_rows, n_cols], f32, tag="misc", space="PSUM")
                for och in range(n_cols):
                    nc.tensor.matmul(pst[:, och:och + 1], lhsT(och), rhs,
                                     start=True, stop=True)
                if k == 0:
                    nc.vector.tensor_copy(acc[:], pst[:])
                else:
                    nc.vector.tensor_tensor(acc[:], acc[:], pst[:], op=ADD)
            o = sb.tile([p_rows, n_cols], f32, tag=out_tag)
            for och in range(n_cols):
                nc.scalar.activation(o[:, och:och + 1], acc[:, och:och + 1],
                                     RELU, bias=act_bias[:, och:och + 1])
            yield o

        def run_chain(p_rows, n_cols, pieces, act_bias, out_tag):
            """pieces: list of (lhsT_fn(och), rhs_ap)."""
            gen = mm_chain(p_rows, n_cols, len(pieces), act_bias, out_tag)
            k = next(gen)
            while True:
                r = gen.send(pieces[k])
                if not isinstance(r, int):
                    return r
                k = r

        wlb0 = load(wp, "Wlb", rows=128, tag="wlb0")
        wlb1 = wp.tile([P, 128], f32, tag="wlb1")
        nc.sync.dma_start(wlb1[0:72, :], io["Wlb"][128:200, :])
        blb = load(wp, "blb_col")
        bvec = run_chain(P, 1, [
            (lambda o: wlb0[:, 0:128], bgc0[:]),
            (lambda o: wlb1[0:72, 0:128], bgc1[0:72, :]),
        ], blb, "bvec")

        wd1_0 = load(wp, "Wd1", rows=128, tag="wd10")
        wd1_1 = wp.tile([P, 500], f32, tag="wd11")
        nc.sync.dma_start(wd1_1[:], io["Wd1"][128:256, :])
        bd1 = load(wp, "bd1_t")
        d1 = run_chain(125, 4, [
            (lambda o: wd1_0[:, o * 125:(o + 1) * 125], mvec[:]),
            (lambda o: wd1_1[:, o * 125:(o + 1) * 125], bvec[:]),
        ], bd1, "d1")

        wd2t = [wp.tile([125, 256], f32, tag=f"wd2_{k}", name=f"wd2_{k}")
                for k in range(4)]
        for k in range(4):
            nc.sync.dma_start(wd2t[k][:], io["Wd2"][k * 125:(k + 1) * 125, :])
        bd2 = load(wp, "bd2_t")
        d2 = run_chain(P, 2, [
            (lambda o, k=k: wd2t[k][:, o * P:(o + 1) * P], d1[:, k:k + 1])
            for k in range(4)
        ], bd2, "d2")

        bl1c = load(wp, "bl1c_t")
        c1 = sb.tile([125, 8], f32, tag="c1")
        for j in range(8):
            tmpc = sb.tile([125, 1], f32, tag="ctmp")
            nc.sync.dma_start(tmpc[:], ar_out.ap()[j * 125:(j + 1) * 125, None])
            nc.scalar.activation(c1[:, j:j + 1], tmpc[:], RELU,
                                 bias=bl1c[:, j:j + 1])

        bl2c = load(wp, "bl2c_t")
        c2 = run_chain(125, 8, [
            (lambda o, k=k: wtc2[k][:, o * 125:(o + 1) * 125], c1[:, k:k + 1])
            for k in range(8)
        ], bl2c, "c2")

        bl3c = load(wp, "bl3c_t")
        c3 = run_chain(P, 2, [
            (lambda o, k=k: wtc3[k][:, o * P:(o + 1) * P], c2[:, k:k + 1])
            for k in range(8)
        ], bl3c, "c3")

        bcat1 = load(wp, "bcat1_t")
        rhs_cat = [d2[:, 0:1], d2[:, 1:2], c3[:, 0:1], c3[:, 1:2]]
        u = run_chain(125, 8, [
            (lambda o, k=k: wtu[k][:, o * 125:(o + 1) * 125], rhs_cat[k])
            for k in range(4)
        ], bcat1, "u")

        wcat2 = wp.tile([125, 8], f32, tag="wcat2")
        for k in range(8):
            nc.sync.dma_start(wcat2[:, k:k + 1],
                              io["Wcat2"][k * 125:(k + 1) * 125, 0:1])
        bcat2 = load(wp, "bcat2_t")
        pso = psM.tile([1, 1], f32, tag="misc", space="PSUM")
        for k in range(8):
            nc.tensor.matmul(pso[:], wcat2[:, k:k + 1], u[:, k:k + 1],
                             start=(k == 0), stop=(k == 7))
        osb = sb.tile([1, 1], f32, tag="osb")
        nc.scalar.activation(osb[:], pso[:], RELU, bias=bcat2[:])
        nc.sync.dma_start(out[0:1, 0:1], osb[:])


# ------------------------------------------------------------------- entry

_CACHE = {}


def kernel(**inputs):
    in_maps, meta = prep_inputs(inputs)
    key = tuple(meta["ctlo"])
    if key not in _CACHE:
        _CACHE[key] = build_program(meta)
    nc = _CACHE[key]
    res = run_bass_kernel_spmd(nc, in_maps, core_ids=list(range(NCORES)))
    return np.asarray(res.results[0]["out"], np.float32)
